# revision 34
# baseline (speedup 1.0000x reference)
"""Attention-gate block (conv1x1+BN x2 -> relu -> conv1x1+BN -> sigmoid -> mul)
on 8 TRN2 NeuronCores, data-parallel over batch with sync-BN via tiny AllGathers.

Self-contained: hardcodes shapes B=16, C=256, F=128, H=W=64, 8 cores.

v3: bf16 datapath (SWDGE cast loads consolidated to 6 ops to keep the Q7
descriptor-generator ahead of the collective triggers; f32 BN stats from
PSUM; f32 sync stores), psi conv packed to PSUM rows {0,32,64,96} via
tile_position, k-split conv issue order for a short post-load tail, shared
broadcast matmuls, DVE 2x/4x modes, small DMAs on the scalar HWDGE ring.
"""
import numpy as np

import concourse.bacc as bacc
import concourse.mybir as mybir
import concourse.tile as tile
from concourse.bass_utils import run_bass_kernel_spmd

F32 = mybir.dt.float32
F32R = mybir.dt.float32r
BF16 = mybir.dt.bfloat16
AF = mybir.ActivationFunctionType
OP = mybir.AluOpType

N_CORES = 8
B, C, F, HW = 16, 256, 128, 64 * 64        # full batch, channels, F_int, pixels/sample
SPC = B // N_CORES                          # samples per core = 2
NPIX = SPC * HW                             # pixels per core = 8192
NTOT = B * HW                               # global BN count = 65536
EPS = 1e-5
NT = HW // 512                              # 512-pixel tiles per sample = 8


def _build():
    nc = bacc.Bacc(trn_type="TRN2", target_bir_lowering=False, debug=False,
                   num_devices=N_CORES)
    g_d = nc.dram_tensor("g_sh", [SPC, C, HW], F32, kind="ExternalInput")
    x_d = nc.dram_tensor("x_sh", [SPC, C, HW], F32, kind="ExternalInput")
    wgT_d = nc.dram_tensor("wgT", [C, F], F32, kind="ExternalInput")
    wxT_d = nc.dram_tensor("wxT", [C, F], F32, kind="ExternalInput")
    psiw_d = nc.dram_tensor("psiw", [F, 1], F32, kind="ExternalInput")
    gb_d = nc.dram_tensor("gb", [F, 4], F32, kind="ExternalInput")
    psigb_d = nc.dram_tensor("psigb", [1, 4], F32, kind="ExternalInput")
    out_d = nc.dram_tensor("out_sh", [SPC, C, HW], F32, kind="ExternalOutput")

    with tile.TileContext(nc) as tc:
        _body(nc, tc, g_d, x_d, wgT_d, wxT_d, psiw_d, gb_d, psigb_d, out_d)
    nc.finalize()
    return nc


def _body(nc, tc, g_d, x_d, wgT_d, wxT_d, psiw_d, gb_d, psigb_d, out_d):
    from contextlib import ExitStack
    es = ExitStack()
    const = es.enter_context(tc.tile_pool(name="const", bufs=1))
    xpool = es.enter_context(tc.tile_pool(name="xdata", bufs=1))
    gpool = es.enter_context(tc.tile_pool(name="gdata", bufs=2))
    ypool = es.enter_context(tc.tile_pool(name="ydata", bufs=1))
    spool = es.enter_context(tc.tile_pool(name="stats", bufs=1))
    zpool = es.enter_context(tc.tile_pool(name="zscratch", bufs=3))
    upool = es.enter_context(tc.tile_pool(name="udata", bufs=1))
    opool = es.enter_context(tc.tile_pool(name="osb", bufs=3))
    dram = es.enter_context(tc.tile_pool(name="drambounce", bufs=1, space="DRAM"))

    ROWS = slice(0, 128, 32)                 # psum-packed psi rows {0,32,64,96}

    # ---- DRAM bounce buffers for collectives ----
    wu_in = dram.tile([1, 8], F32, tag="wu_in")
    wu_out = dram.tile([8, 8], F32, tag="wu_out")
    in_b = dram.tile([128, 4], F32, tag="ar1_in")
    out_b = dram.tile([1024, 4], F32, tag="ar1_out")
    in2_b = dram.tile([4, 8], F32, tag="ar2_in")
    out2_b = dram.tile([32, 8], F32, tag="ar2_out")

    # ---- constants / weights ----
    wu_sb = spool.tile([1, 8], F32, tag="wu_sb")
    nc.vector.memset(wu_sb[:], 0.0)
    # ncfw warm-up: first collective pays ~25us init; fire it before anything
    # else on the gpsimd queue so it's long done before the stats AllGather.
    # The gpsimd queue carries ONLY the 6 input loads + collective triggers:
    # each SWDGE dma_start costs ~5us of Q7 descriptor-gen, so everything else
    # (weights, bounce DMAs, stores) goes via the HWDGE rings (sync/scalar).
    nc.scalar.dma_start(wu_in[:], wu_sb[:])
    nc.gpsimd.collective_compute("AllGather", OP.bypass,
                                 replica_groups=[list(range(N_CORES))],
                                 ins=[wu_in.opt()], outs=[wu_out.opt()])

    w_gf = const.tile([128, 2, 128], F32, tag="w_gf")
    w_xf = const.tile([128, 2, 128], F32, tag="w_xf")
    for k in range(2):
        nc.sync.dma_start(w_gf[:, k, :], wgT_d[k * 128:(k + 1) * 128, :])
        nc.sync.dma_start(w_xf[:, k, :], wxT_d[k * 128:(k + 1) * 128, :])
    w_g = const.tile([128, 2, 128], BF16, tag="w_g")
    nc.scalar.activation(w_g[:], w_gf[:], AF.Copy)
    w_x = const.tile([128, 2, 128], BF16, tag="w_x")
    nc.scalar.activation(w_x[:], w_xf[:], AF.Copy)
    # warm the sqrt act-table while idle: sqrt_and_others also holds
    # copy/square/relu, so the only table swap left is the final sigmoid
    dum = const.tile([1, 1], F32, tag="dum")
    nc.scalar.activation(dum[:], wu_sb[0:1, 0:1], AF.Sqrt)
    psiw = const.tile([128, 1], F32, tag="psiw")
    nc.sync.dma_start(psiw[:], psiw_d[:])
    gb = const.tile([128, 4], F32, tag="gb")
    nc.sync.dma_start(gb[:], gb_d[:])
    psigb = const.tile([1, 4], F32, tag="psigb")
    nc.sync.dma_start(psigb[:], psigb_d[:])
    ones_bc = const.tile([128, 128], BF16, tag="ones_bc")
    nc.vector.memset(ones_bc[:], 1.0)
    ones_r = const.tile([1, 128], F32, tag="ones_r")
    nc.vector.memset(ones_r[:], 1.0)
    epsc = const.tile([128, 1], F32, tag="epsc")
    nc.vector.memset(epsc[:], EPS)

    # ---- persistent big buffers ----
    x_t = [xpool.tile([128, 2, HW], BF16, tag=f"x_{s}", name=f"x_{s}")
           for s in range(SPC)]
    y_g = ypool.tile([128, NPIX], BF16, tag="y_g")
    y_x = ypool.tile([128, NPIX], BF16, tag="y_x")

    acc_bn_g = spool.tile([128, SPC * NT * 6], F32, tag="abn_g")
    acc_bn_x = spool.tile([128, SPC * NT * 6], F32, tag="abn_x")
    bn2 = spool.tile([128, 4], F32, tag="bn2")      # (mean,var) g|x from bn_aggr
    S_g = spool.tile([128, 2], F32, tag="S_g")
    S_x = spool.tile([128, 2], F32, tag="S_x")
    R = spool.tile([128, 4], F32, tag="R")          # allreduced S1g,S2g,S1x,S2x
    gth = spool.tile([128, 8, 4], F32, tag="gth")
    pv = spool.tile([128, 16], F32, tag="pv")       # param scratch columns
    prm = spool.tile([128, 4], F32, tag="prm")      # s_g, s_x, tsum, q
    # psi weights in col 0, zeros in cols 1-31: the psi matmul then writes a
    # full 32-row PSUM block (row r = u chunk, rows r+1..r+31 = 0), so later
    # compute ops can use full-partition APs (ACT/DVE reject partition strides)
    psiw2 = spool.tile([128, 32], BF16, tag="psiw2")
    nc.vector.memset(psiw2[:], 0.0)

    # psi conv output: chunk t (512 px) at row 32*(t%4), col-block t//4
    u_sb = upool.tile([128, 4 * 512], F32, tag="u_sb")
    psi_sb = upool.tile([128, 4 * 512], BF16, tag="psi_sb")
    usum = spool.tile([128, 8], F32, tag="usum")    # per-pu sum (cols 0-3) / sumsq (4-7)
    ujunk = upool.tile([128, 512], BF16, tag="ujunk")
    gth2 = spool.tile([1, 256], F32, tag="gth2")
    r2 = spool.tile([1, 8], F32, tag="r2")
    pm = spool.tile([1, 8], F32, tag="pm")
    ab = spool.tile([1, 2], F32, tag="ab")          # A, B scalars
    ab128 = spool.tile([128, 2], F32, tag="ab128")

    # ================= Phase A: load + conv matmuls + local stats =================
    # SWDGE (gpsimd) loads with f32->bf16 cast inline, consolidated into 6 ops
    # so Q7 stays ahead of the DMA stream; g1 is last and chunked fine so only
    # ~4 matmuls trail the final DMA byte.
    g_t = {}
    gt0 = gpool.tile([128, 2, HW], BF16, tag="gld", name="g_0")
    nc.gpsimd.dma_start(gt0[:], g_d[0].rearrange("(k c) p -> c k p", k=2))
    g_t[0] = gt0
    for s in range(SPC):
        nc.gpsimd.dma_start(x_t[s][:], x_d[s].rearrange("(k c) p -> c k p", k=2))
    gt1 = gpool.tile([128, 2, HW], BF16, tag="gld", name="g_1")
    nc.gpsimd.dma_start(gt1[:, 0, :], g_d[1, 0:128, :])
    for hh in range(2):
        hs = slice(hh * (HW // 2), (hh + 1) * (HW // 2))
        nc.gpsimd.dma_start(gt1[:, 1, hs], g_d[1, 128:256, hs])
    g_t[1] = gt1

    # conv order: (s0,g),(s0,x),(s1,x),(s1,g) to match DMA arrival order.
    # Within a conv: all k=0 matmuls (one weight load), then all k=1.
    with tc.tile_pool(name="psumA", bufs=8, space="PSUM") as psA:
        for s, tname, wt, src, ysb, abn in (
                (0, "g", w_g, g_t[0], y_g, acc_bn_g),
                (0, "x", w_x, x_t[0], y_x, acc_bn_x),
                (1, "x", w_x, x_t[1], y_x, acc_bn_x),
                (1, "g", w_g, g_t[1], y_g, acc_bn_g)):
            ps = [psA.tile([128, 512], F32, tag="psA", name=f"ps_{s}{tname}_{j}")
                  for j in range(NT)]
            for k in range(2):
                for j in range(NT):
                    js = slice(j * 512, (j + 1) * 512)
                    nc.tensor.matmul(ps[j][:], wt[:, k, :], src[:, k, js],
                                     start=(k == 0), stop=(k == 1))
            for j in range(NT):
                ys = slice(s * HW + j * 512, s * HW + (j + 1) * 512)
                nc.scalar.activation(ysb[:, ys], ps[j][:], AF.Copy)
                t = s * NT + j
                nc.vector.bn_stats(abn[:, t * 6:(t + 1) * 6], ps[j][:])

    # local (mean,var) -> (S1,S2); one combined AllGather, local reduce
    for i, (abn, Sp) in enumerate(((acc_bn_x, S_x), (acc_bn_g, S_g))):
        nc.vector.bn_aggr(bn2[:, 2 * i:2 * i + 2], abn[:])
        m = bn2[:, 2 * i:2 * i + 1]
        v = bn2[:, 2 * i + 1:2 * i + 2]
        nc.vector.tensor_scalar(Sp[:, 0:1], m, float(NPIX), None, OP.mult)
        nc.vector.scalar_tensor_tensor(pv[:, 15:16], m, m, v, OP.mult, OP.add)
        nc.vector.tensor_scalar(Sp[:, 1:2], pv[:, 15:16], float(NPIX), None, OP.mult)
    nc.sync.dma_start(in_b[:, 2:4], S_x[:])
    nc.sync.dma_start(in_b[:, 0:2], S_g[:])
    nc.gpsimd.collective_compute("AllGather", OP.bypass,
                                 replica_groups=[list(range(N_CORES))],
                                 ins=[in_b.opt()], outs=[out_b.opt()])
    nc.sync.dma_start(gth[:], out_b[:].rearrange("(r c) f -> c r f", c=128))
    nc.vector.tensor_reduce(R[:], gth[:].rearrange("c r f -> c f r"),
                            mybir.AxisListType.X, OP.add)

    # ================= Phase B: per-channel affine params (2-col vectorized) ====
    # R cols: 0=S1x 1=S2x 2=S1g 3=S2g... NOTE: in_b cols 0:2=S_g, 2:4=S_x, so
    # R cols 0:2 are g (S1,S2), 2:4 are x.  pv cols: 0:2 means (g,x), 2:4 msqs,
    # 4:6 -var, 6:8 std, 8:10 inv, 10:12 t, 12 inv_sg, 13 c, 14 psiw*s_g
    invn = 1.0 / float(NTOT)
    nc.vector.tensor_scalar(pv[:, 0:2], R[:, 0:4:2], invn, None, OP.mult)
    nc.vector.tensor_scalar(pv[:, 2:4], R[:, 1:4:2], invn, None, OP.mult)
    nc.vector.tensor_mul(pv[:, 4:6], pv[:, 0:2], pv[:, 0:2])
    nc.vector.tensor_sub(pv[:, 4:6], pv[:, 4:6], pv[:, 2:4])    # mean^2-msq = -var
    nc.scalar.activation(pv[:, 6:8], pv[:, 4:6], AF.Sqrt, bias=epsc[:], scale=-1.0)
    nc.vector.reciprocal(pv[:, 8:10], pv[:, 6:8])
    nc.vector.tensor_mul(prm[:, 0:2], pv[:, 8:10], gb[:, 0:4:2])  # s_g, s_x
    nc.vector.tensor_mul(pv[:, 10:12], pv[:, 0:2], prm[:, 0:2])
    nc.vector.tensor_sub(pv[:, 10:12], gb[:, 1:4:2], pv[:, 10:12])  # t = beta-mean*s
    nc.vector.tensor_add(prm[:, 2:3], pv[:, 10:11], pv[:, 11:12])   # tsum
    # rescale trick (valid for s_g > 0, i.e. wg_gamma > 0, checked on host):
    # z' = y_g + q*y_x + c with q = s_x/s_g, c = tsum/s_g; relu(z) = s_g*relu(z'),
    # s_g folded into psi weights.
    nc.vector.reciprocal(pv[:, 12:13], prm[:, 0:1])                # 1/s_g
    nc.vector.tensor_mul(prm[:, 3:4], prm[:, 1:2], pv[:, 12:13])   # q
    nc.vector.tensor_mul(pv[:, 13:14], prm[:, 2:3], pv[:, 12:13])  # c
    nc.vector.tensor_mul(pv[:, 14:15], psiw[:], prm[:, 0:1])
    nc.scalar.activation(psiw2[:, 0:1], pv[:, 14:15], AF.Copy)     # -> bf16

    # ================= Phase C: z' = y_g + q*y_x; relu(+c); psi conv =================
    # relu output (bf16) reuses the dead g-tile slots
    r_t = gpool.tile([128, 2, HW], BF16, tag="gld", name="r_t")
    pu = {}
    with tc.tile_pool(name="psumU", bufs=2, space="PSUM") as psU:
        for s in range(SPC):
            for j in range(4):                 # 1024-px blocks
                js = slice(s * HW + j * 1024, s * HW + (j + 1) * 1024)
                rs = slice(j * 1024, (j + 1) * 1024)
                z = zpool.tile([128, 1024], BF16, tag="z")
                nc.vector.tensor_scalar(z[:], y_x[:, js], prm[:, 3:4], None, OP.mult)
                nc.vector.tensor_add(z[:], z[:], y_g[:, js])
                # alternate relu between DVE (4x tensor_scalar) and ACT to
                # balance the two queues; the +c fold rides along either way
                if j % 2 == 0:
                    nc.scalar.activation(r_t[:, s, rs], z[:], AF.Relu,
                                         bias=pv[:, 13:14])
                else:
                    nc.vector.tensor_scalar(r_t[:, s, rs], z[:], pv[:, 13:14],
                                            0.0, OP.add, OP.max)
                for v in range(2):
                    t = s * 8 + 2 * j + v
                    b, r = t // 4, 32 * (t % 4)
                    if r == 0:
                        pu[b] = psU.tile([128, 512], F32, tag="psU", name=f"pu_{b}")
                    cs = slice((2 * j + v) * 512, (2 * j + v + 1) * 512)
                    nc.tensor.matmul(pu[b][r:r + 32, :], psiw2[:], r_t[:, s, cs],
                                     start=True, stop=True, tile_position=(0, r))
                    if r == 96:
                        bs = slice(b * 512, (b + 1) * 512)
                        nc.scalar.activation(u_sb[:, bs], pu[b][:],
                                             AF.Copy, accum_out=usum[:, b:b + 1])
                        if b == 3:
                            # last block: square on the (now idle) DVE so the
                            # AG2 trigger isn't stuck behind the scalar queue
                            # (u_sb copy x psum value; DVE allows one PSUM src)
                            nc.vector.scalar_tensor_tensor(
                                ujunk[:], u_sb[:, bs], 1.0, pu[b][:],
                                OP.mult, OP.mult, accum_out=usum[:, 7:8])
                        else:
                            nc.scalar.activation(ujunk[:], pu[b][:],
                                                 AF.Square,
                                                 accum_out=usum[:, 4 + b:5 + b])

    # ================= AllGather 2: psi stats, local reduce =================
    nc.sync.dma_start(in2_b[:], usum[ROWS, :])
    nc.gpsimd.collective_compute("AllGather", OP.bypass,
                                 replica_groups=[list(range(N_CORES))],
                                 ins=[in2_b.opt()], outs=[out2_b.opt()])
    nc.sync.dma_start(gth2[:], out2_b[:].rearrange("r f -> (r f)"))
    # 256 = 32 rows x (4 sum + 4 sumsq) cols -> [1,8] -> [1,2]
    nc.vector.tensor_reduce(r2[:], gth2[:].rearrange("p (n c) -> p c n", c=8),
                            mybir.AxisListType.X, OP.add)
    nc.vector.tensor_reduce(pm[0:1, 0:2], r2[0:1, :].rearrange("p (a b) -> p a b", a=2),
                            mybir.AxisListType.X, OP.add)

    # psi affine scalars A, B on partition 0
    nc.vector.tensor_scalar(pm[0:1, 2:4], pm[0:1, 0:2], invn, None, OP.mult)
    nc.vector.tensor_mul(pm[0:1, 4:5], pm[0:1, 2:3], pm[0:1, 2:3])
    nc.vector.tensor_sub(pm[0:1, 4:5], pm[0:1, 4:5], pm[0:1, 3:4])   # -var
    nc.scalar.activation(pm[0:1, 5:6], pm[0:1, 4:5], AF.Sqrt,
                         bias=epsc[0:1, :], scale=-1.0)
    nc.vector.reciprocal(pm[0:1, 6:7], pm[0:1, 5:6])
    nc.vector.tensor_mul(ab[0:1, 0:1], pm[0:1, 6:7], psigb[0:1, 0:1])  # A
    nc.vector.tensor_mul(pm[0:1, 7:8], pm[0:1, 2:3], ab[0:1, 0:1])
    nc.vector.tensor_sub(ab[0:1, 1:2], psigb[0:1, 1:2], pm[0:1, 7:8])  # B

    # ================= Phase E: sigmoid, broadcast, multiply, store =================
    with tc.tile_pool(name="psumAB", bufs=1, space="PSUM") as psABp, \
         tc.tile_pool(name="psumB", bufs=3, space="PSUM") as psB:
        # broadcast (A,B) to all partitions via a tiny fp32 matmul (PE is idle
        # and this avoids a slow gpsimd partition_broadcast on the Q7)
        psab = psABp.tile([128, 2], F32, tag="psab")
        nc.tensor.matmul(psab[:], ones_r[:], ab[0:1, :], start=True, stop=True)
        nc.scalar.activation(ab128[:], psab[:], AF.Copy)
        # sigmoid split per sample (cols 0:1024 = sample 0's chunks) so the
        # first broadcast matmuls don't wait for the full table
        for half in range(2):
            hs = slice(half * 1024, (half + 1) * 1024)
            nc.scalar.activation(psi_sb[:, hs], u_sb[:, hs], AF.Sigmoid,
                                 bias=ab128[:, 1:2], scale=ab128[:, 0:1])

        for s in range(SPC):
            o = [opool.tile([128, HW], F32, tag="o", name=f"o_{s}_{k}")
                 for k in range(2)]
            for jh in range(2):                # halves of the sample (2048 px)
                pbt = {}
                for j in (2 * jh, 2 * jh + 1):     # two 1024-px blocks
                    pb = pbt[j] = psB.tile([128, 1024], F32, tag="pb",
                                           name=f"pb_{s}_{j}")
                    for v in range(2):
                        t = s * 8 + 2 * j + v
                        b, r = t // 4, 32 * (t % 4)
                        bs = slice(b * 512, (b + 1) * 512)
                        nc.tensor.matmul(pb[:, v * 512:(v + 1) * 512],
                                         ones_bc[r:r + 1, :], psi_sb[r:r + 1, bs],
                                         start=True, stop=True,
                                         tile_position=(r, 0))
                # k-major muls: each (s,k) half is complete after two muls, so
                # its 1MB store (8KB/partition descriptors) streams immediately
                hs = slice(2 * jh * 1024, (2 * jh + 2) * 1024)
                for k in range(2):
                    for j in (2 * jh, 2 * jh + 1):
                        xs = slice(j * 1024, (j + 1) * 1024)
                        nc.vector.tensor_mul(o[k][:, xs], x_t[s][:, k, xs],
                                             pbt[j][:])
                    nc.sync.dma_start(
                        out_d[s, k * 128:(k + 1) * 128, hs], o[k][:, hs])
    es.close()


_NC_CACHE = []


def _numpy_ref(ins):
    g = np.asarray(ins["g"], np.float64)
    x = np.asarray(ins["x"], np.float64)

    def conv(v, w, b):
        return np.einsum("bchw,oc->bohw", v, np.asarray(w, np.float64)) + \
            np.asarray(b, np.float64)[None, :, None, None]

    def bn(v, gam, bet):
        m = v.mean(axis=(0, 2, 3), keepdims=True)
        s = v.var(axis=(0, 2, 3), keepdims=True)
        vh = (v - m) / np.sqrt(s + EPS)
        return vh * np.asarray(gam, np.float64)[None, :, None, None] + \
            np.asarray(bet, np.float64)[None, :, None, None]

    g1 = bn(conv(g, ins["wg_w"], ins["wg_b"]), ins["wg_gamma"], ins["wg_beta"])
    x1 = bn(conv(x, ins["wx_w"], ins["wx_b"]), ins["wx_gamma"], ins["wx_beta"])
    p = np.maximum(g1 + x1, 0.0)
    p = bn(conv(p, ins["psi_w"], ins["psi_b"]), ins["psi_gamma"], ins["psi_beta"])
    p = 1.0 / (1.0 + np.exp(-p))
    return (x * p).astype(np.float32)


def _prep_inputs(inputs):
    g = np.ascontiguousarray(np.asarray(inputs["g"], np.float32)).reshape(B, C, HW)
    x = np.ascontiguousarray(np.asarray(inputs["x"], np.float32)).reshape(B, C, HW)
    wgT = np.ascontiguousarray(np.asarray(inputs["wg_w"], np.float32).T)
    wxT = np.ascontiguousarray(np.asarray(inputs["wx_w"], np.float32).T)
    psiw = np.ascontiguousarray(np.asarray(inputs["psi_w"], np.float32).reshape(1, F).T)
    gb = np.ascontiguousarray(np.stack([
        np.asarray(inputs["wg_gamma"], np.float32),
        np.asarray(inputs["wg_beta"], np.float32),
        np.asarray(inputs["wx_gamma"], np.float32),
        np.asarray(inputs["wx_beta"], np.float32)], axis=1))
    psigb = np.array([[float(np.asarray(inputs["psi_gamma"]).reshape(-1)[0]),
                       float(np.asarray(inputs["psi_beta"]).reshape(-1)[0]), 0.0, 0.0]],
                     np.float32)
    in_maps = []
    for i in range(N_CORES):
        sl = slice(i * SPC, (i + 1) * SPC)
        in_maps.append({"g_sh": g[sl], "x_sh": x[sl], "wgT": wgT, "wxT": wxT,
                        "psiw": psiw, "gb": gb, "psigb": psigb})
    return in_maps


def kernel(**inputs):
    if np.any(np.asarray(inputs["wg_gamma"]) <= 0):
        # device kernel folds s_g>0 into relu; fall back if precondition broken
        return _numpy_ref(inputs)
    if not _NC_CACHE:
        _NC_CACHE.append(_build())
    nc = _NC_CACHE[0]
    in_maps = _prep_inputs(inputs)
    res = run_bass_kernel_spmd(nc, in_maps, list(range(N_CORES)))
    out = np.concatenate([res.results[i]["out_sh"] for i in range(N_CORES)], axis=0)
    return out.reshape(B, C, 64, 64)


def run_traced(**inputs):
    """Like kernel() but with NTFF tracing; returns (out, BassKernelResults)."""
    if not _NC_CACHE:
        _NC_CACHE.append(_build())
    nc = _NC_CACHE[0]
    in_maps = _prep_inputs(inputs)
    res = run_bass_kernel_spmd(nc, in_maps, list(range(N_CORES)), trace=True,
                               trace_cores=list(range(N_CORES)))
    out = np.concatenate([res.results[i]["out_sh"] for i in range(N_CORES)], axis=0)
    return out.reshape(B, C, 64, 64), res
